# revision 8
# baseline (speedup 1.0000x reference)
"""Trainium2 Bass kernel for nn_Attention_76089640616322.

Bahdanau-style attention:
  B, S, HE, DOUT = 32, 4096, 512, 512  (HD = 1024)
  energy = tanh(concat([context, broadcast(output)], -1) @ W1.T)   [B,S,HE]
  attn   = softmax(energy @ W2.T, axis=S)                           [B,1,S]
  mix    = attn @ context                                           [B,1,HE]
  out    = tanh(concat([mix, output], -1) @ Wout.T + bout)          [B,1,HE]

Sharding: pure data parallel, batch dim across 8 cores (4 batches/core),
weights replicated.

Key algebraic restructure: the broadcast `output` columns of the concat make
W1 @ concat(...) = W1[:, :HE] @ context + (W1[:, HE:] @ output_b), the second
term a per-batch constant -> fused as a per-partition bias into the tanh.
Softmax is computed unnormalized (|logit| <= ||W2||_1 ~ 8, exp is safe in
fp32) with the normalization folded in after the mix contraction.

All matmuls run as float32r (full-rate TF32-like PE mode; fp32 storage).
The walrus verifier requires f32r matmul operands to come from f32r-typed
producers, so input DRAM tensors and all SBUF tiles on the matmul paths are
declared float32r; PSUM accumulators of regular matmuls stay fp32.
"""

from contextlib import ExitStack

import numpy as np

import concourse.bass as bass
import concourse.tile as tile
from concourse import bacc, mybir
from concourse._compat import with_exitstack
from concourse.masks import make_identity

B, S, HE, DOUT = 32, 4096, 512, 512
HD = HE + DOUT
NCORES = 8
BC = B // NCORES  # batches per core

F32 = mybir.dt.float32
F32R = mybir.dt.float32r
AF = mybir.ActivationFunctionType

NSBLK = 8       # s-blocks per batch (512 s each)
SBLK = S // NSBLK   # 512
NSS = SBLK // 128   # 4 subtiles of 128 s per block
NEC = HE // 128     # 4 e-chunks
NDC = HE // 128     # 4 d-chunks for the context half of W1
SCHUNKS = S // 128  # 32 s-chunks of 128 per batch


@with_exitstack
def attention_kernel(ctx: ExitStack, tc: tile.TileContext, out_ap, ins):
    nc = tc.nc

    ctx_ap = ins["context"]    # [BC, S, HE]  f32r
    outp_ap = ins["output"]    # [BC, 1, DOUT] f32r
    w1_ap = ins["W1"]          # [HE, HD] f32r
    w2_ap = ins["W2"]          # [1, HE] f32r
    wout_ap = ins["Wout"]      # [HE, HD] f32r
    bout_ap = ins["bout"]      # [HE] f32r

    const = ctx.enter_context(tc.tile_pool(name="const", bufs=1))
    wload = ctx.enter_context(tc.tile_pool(name="wload", bufs=1))
    ctx_pool = ctx.enter_context(tc.tile_pool(name="ctx", bufs=9))
    ctxT_pool = ctx.enter_context(tc.tile_pool(name="ctxT", bufs=8))
    tanh_pool = ctx.enter_context(tc.tile_pool(name="tanh", bufs=8))
    small = ctx.enter_context(tc.tile_pool(name="small", bufs=2))

    psum_tp = ctx.enter_context(tc.tile_pool(name="ptp", bufs=2, space="PSUM"))
    psum_en = ctx.enter_context(tc.tile_pool(name="pen", bufs=2, space="PSUM"))
    psum_pcol = ctx.enter_context(tc.tile_pool(name="ppcol", bufs=2, space="PSUM"))
    psum_misc = ctx.enter_context(tc.tile_pool(name="pmisc", bufs=2, space="PSUM"))

    # ---- constants ----
    id128f = const.tile([128, 128], F32)
    make_identity(nc, id128f)
    id128 = const.tile([128, 128], F32R)
    nc.vector.tensor_copy(id128, id128f)
    ones1f = const.tile([1, 1], F32)
    nc.vector.memset(ones1f, 1.0)
    ones1 = const.tile([1, 1], F32R)
    nc.vector.tensor_copy(ones1, ones1f)
    ones128 = const.tile([128, 1], F32)
    nc.vector.memset(ones128, 1.0)

    # ---- load weights (f32r straight from DRAM) ----
    w1_t = w1_ap.rearrange("(c p) d -> c p d", p=128)     # [4,128,1024]
    wout_t = wout_ap.rearrange("(c p) d -> c p d", p=128)
    w1sb = []
    woutsb = []
    for c in range(NEC):
        t1 = wload.tile([128, HD], F32R, tag=f"w1sb{c}")
        nc.sync.dma_start(out=t1, in_=w1_t[c])
        w1sb.append(t1)
        t2 = wload.tile([128, HD], F32R, tag=f"woutsb{c}")
        nc.sync.dma_start(out=t2, in_=wout_t[c])
        woutsb.append(t2)

    w2sb = const.tile([1, HE], F32)
    nc.sync.dma_start(out=w2sb, in_=w2_ap)
    boutsb = const.tile([1, HE], F32)
    nc.sync.dma_start(out=boutsb, in_=bout_ap.rearrange("(a d) -> a d", a=1))
    outp_rows = []
    for b in range(BC):
        t = const.tile([1, DOUT], F32, tag=f"outp_row{b}")
        nc.sync.dma_start(out=t, in_=outp_ap[b])
        outp_rows.append(t)

    # ---- transpose W1 -> W1T (8 tiles [d=128, e=512]) and Wout -> WoutT ----
    w1T = []
    woutT = []
    for name, src, dstlist in (("w1T", w1sb, w1T), ("woutT", woutsb, woutT)):
        for dc in range(HD // 128):
            ps = psum_tp.tile([128, HE], F32R, tag="tp")
            for ec in range(NEC):
                nc.tensor.transpose(
                    ps[:, ec * 128:(ec + 1) * 128],
                    src[ec][:, dc * 128:(dc + 1) * 128],
                    id128,
                )
            dst = const.tile([128, HE], F32R, tag=f"{name}{dc}")
            nc.vector.tensor_copy(dst, ps)
            dstlist.append(dst)

    # ---- columnize W2 (zero-padded pairs for f32r stationary), bout, output ----
    ps = psum_misc.tile([128, 2 * NEC], F32, tag="misc")
    nc.vector.memset(ps, 0.0)
    for ec in range(NEC):
        nc.tensor.transpose(
            ps[:, 2 * ec:2 * ec + 1], w2sb[:, ec * 128:(ec + 1) * 128], ones1f
        )
    w2col2 = const.tile([128, 2 * NEC], F32R)
    nc.vector.tensor_copy(w2col2, ps)

    ps = psum_misc.tile([128, NEC], F32, tag="misc")
    for ec in range(NEC):
        nc.tensor.transpose(
            ps[:, ec:ec + 1], boutsb[:, ec * 128:(ec + 1) * 128], ones1f
        )
    boutcol = const.tile([128, NEC], F32)
    nc.vector.tensor_copy(boutcol, ps)

    # output_b columns: outpcol[:, b*4+dc] = output[b, dc*128 + p]
    ps = psum_misc.tile([128, BC * 4], F32, tag="misc")
    for b in range(BC):
        for dc in range(4):
            nc.tensor.transpose(
                ps[:, b * 4 + dc: b * 4 + dc + 1],
                outp_rows[b][:, dc * 128:(dc + 1) * 128],
                ones1f,
            )
    outpcol = const.tile([128, BC * 4], F32)
    nc.vector.tensor_copy(outpcol, ps)

    # ---- per-batch tanh offsets: off[b] = W1[:, HE:] @ output_b ----
    ps = psum_misc.tile([128, BC * NEC], F32, tag="misc")
    for b in range(BC):
        for ec in range(NEC):
            for dco in range(4):
                nc.tensor.matmul(
                    ps[:, b * NEC + ec: b * NEC + ec + 1],
                    lhsT=w1T[4 + dco][:, ec * 128:(ec + 1) * 128].bitcast(F32),
                    rhs=outpcol[:, b * 4 + dco: b * 4 + dco + 1],
                    start=(dco == 0),
                    stop=(dco == 3),
                )
    offsb = const.tile([128, BC * NEC], F32)
    nc.vector.tensor_copy(offsb, ps)

    # ---- main loop over batches ----
    for b in range(BC):
        ctx_b = ctx_ap[b].rearrange("(k ss p) d -> k p ss d", ss=NSS, p=128)
        ctx_tiles = []
        pcol = psum_pcol.tile([128, 2 * SCHUNKS], F32)
        nc.vector.memset(pcol, -100000.0)

        for k in range(NSBLK):
            # load one s-block [128, 4, 512] (s = k*512 + ss*128 + p)
            ct = ctx_pool.tile([128, NSS, HE], F32R, tag="ctx")
            nc.sync.dma_start(out=ct, in_=ctx_b[k])
            ctx_tiles.append(ct)

            # transpose to ctxT [d=128, s=512] per d-chunk
            ctxT = []
            for dc in range(NDC):
                pt = psum_tp.tile([128, SBLK], F32R, tag="tp")
                for ss in range(NSS):
                    nc.tensor.transpose(
                        pt[:, ss * 128:(ss + 1) * 128],
                        ct[:, ss, dc * 128:(dc + 1) * 128],
                        id128,
                    )
                st = ctxT_pool.tile([128, SBLK], F32R, tag="ctxT")
                nc.vector.tensor_copy(st, pt)
                ctxT.append(st)

            # energyT[e_chunk, s_blk] = sum_dc W1cT[dc,ec].T @ ctxT[dc]
            tanh_tiles = []
            for ec in range(NEC):
                pe = psum_en.tile([128, SBLK], F32, tag="en")
                for dc in range(NDC):
                    nc.tensor.matmul(
                        pe,
                        lhsT=w1T[dc][:, ec * 128:(ec + 1) * 128],
                        rhs=ctxT[dc],
                        start=(dc == 0),
                        stop=(dc == NDC - 1),
                    )
                th = tanh_pool.tile([128, SBLK], F32R, tag="tanh")
                nc.scalar.activation(
                    th, pe, AF.Tanh, bias=offsb[:, b * NEC + ec: b * NEC + ec + 1]
                )
                tanh_tiles.append(th)

            # logits row [1, 512] = sum_ec W2col[ec].T @ tanhT[ec]
            plg = psum_misc.tile([2, SBLK], F32, tag="misc")
            for ec in range(NEC):
                nc.tensor.matmul(
                    plg,
                    lhsT=w2col2[:, 2 * ec:2 * ec + 2],
                    rhs=tanh_tiles[ec],
                    start=(ec == 0),
                    stop=(ec == NEC - 1),
                )
            lg = small.tile([1, SBLK], F32, tag="lg_sb")
            nc.vector.tensor_copy(lg, plg[0:1, :])

            # transpose logit row chunks into pcol columns (s-chunk index)
            for ss in range(NSS):
                j = k * NSS + ss
                nc.tensor.transpose(
                    pcol[:, 2 * j: 2 * j + 1],
                    lg[:, ss * 128:(ss + 1) * 128],
                    ones1f,
                )

        # exp (unnormalized softmax) + per-partition sums
        pexp = small.tile([128, 2 * SCHUNKS], F32R, tag="pexp")
        rowsum = small.tile([128, 1], F32, tag="rowsum")
        nc.scalar.activation(pexp, pcol, AF.Exp, accum_out=rowsum)

        # denom = sum over partitions (plain fp32 matmul, 1-elem output)
        pd = psum_misc.tile([1, 1], F32, tag="misc")
        nc.tensor.matmul(pd, lhsT=rowsum, rhs=ones128)
        inv = small.tile([1, 1], F32, tag="inv")
        nc.vector.reciprocal(inv, pd)

        # mixu [1, 512] = sum_j pexp[:, j].T @ ctx_tile_j
        pmix = psum_misc.tile([2, HE], F32, tag="misc")
        for j in range(SCHUNKS):
            nc.tensor.matmul(
                pmix,
                lhsT=pexp[:, 2 * j:2 * j + 2],
                rhs=ctx_tiles[j // NSS][:, j % NSS, :],
                start=(j == 0),
                stop=(j == SCHUNKS - 1),
            )
        mix = small.tile([1, HE], F32, tag="mix")
        nc.vector.tensor_scalar_mul(mix, pmix[0:1, :], inv)

        # mix columns [128, 4]
        pmc = psum_misc.tile([128, 4], F32, tag="misc")
        for dc in range(4):
            nc.tensor.transpose(
                pmc[:, dc:dc + 1], mix[:, dc * 128:(dc + 1) * 128], ones1f
            )
        mc = small.tile([128, 4], F32, tag="mc_sb")
        nc.vector.tensor_copy(mc, pmc)

        # final: out_col[ec] = sum_dc WoutT[dc,ec].T @ comb_col[dc]
        pfo = psum_misc.tile([128, NEC], F32, tag="misc")
        for ec in range(NEC):
            for dc in range(8):
                rhs = (
                    mc[:, dc:dc + 1]
                    if dc < 4
                    else outpcol[:, b * 4 + (dc - 4): b * 4 + (dc - 4) + 1]
                )
                nc.tensor.matmul(
                    pfo[:, ec:ec + 1],
                    lhsT=woutT[dc][:, ec * 128:(ec + 1) * 128].bitcast(F32),
                    rhs=rhs,
                    start=(dc == 0),
                    stop=(dc == 7),
                )
        fo = small.tile([128, NEC], F32, tag="fo_sb")
        for ec in range(NEC):
            nc.scalar.activation(
                fo[:, ec:ec + 1], pfo[:, ec:ec + 1], AF.Tanh,
                bias=boutcol[:, ec:ec + 1],
            )

        # back to a row [1, 512] and out
        por = psum_misc.tile([1, HE], F32, tag="misc")
        for ec in range(NEC):
            nc.tensor.transpose(
                por[:, ec * 128:(ec + 1) * 128], fo[:, ec:ec + 1], id128f
            )
        orow = small.tile([1, HE], F32, tag="orow")
        nc.vector.tensor_copy(orow, por)
        nc.sync.dma_start(out=out_ap[b], in_=orow)


INPUT_SPECS = {
    "output": ((BC, 1, DOUT), F32),
    "context": ((BC, S, HE), F32R),
    "W1": ((HE, HD), F32R),
    "W2": ((1, HE), F32),
    "Wout": ((HE, HD), F32R),
    "bout": ((HE,), F32),
}

_CACHE = {}


def build_nc():
    if "nc" in _CACHE:
        return _CACHE["nc"]
    nc = bacc.Bacc("TRN2", target_bir_lowering=False, debug=False,
                   num_devices=NCORES)
    ins = {
        name: nc.dram_tensor(name, list(shape), dt, kind="ExternalInput").ap()
        for name, (shape, dt) in INPUT_SPECS.items()
    }
    out = nc.dram_tensor("out", [BC, 1, HE], F32, kind="ExternalOutput").ap()
    with tile.TileContext(nc) as tc:
        attention_kernel(tc, out, ins)
    nc.compile()
    _CACHE["nc"] = nc
    return nc


def make_in_maps(output, context, W1, W2, Wout, bout):
    maps = []
    for i in range(NCORES):
        sl = slice(i * BC, (i + 1) * BC)
        maps.append({
            "output": np.ascontiguousarray(output[sl], dtype=np.float32),
            "context": np.ascontiguousarray(context[sl], dtype=np.float32),
            "W1": np.ascontiguousarray(W1, dtype=np.float32),
            "W2": np.ascontiguousarray(W2, dtype=np.float32),
            "Wout": np.ascontiguousarray(Wout, dtype=np.float32),
            "bout": np.ascontiguousarray(bout, dtype=np.float32),
        })
    return maps


def run(inputs, trace=False):
    from concourse.bass_utils import run_bass_kernel_spmd

    nc = build_nc()
    in_maps = make_in_maps(**inputs)
    res = run_bass_kernel_spmd(nc, in_maps, list(range(NCORES)), trace=trace)
    out = np.concatenate([res.results[i]["out"] for i in range(NCORES)], axis=0)
    return out, res


def kernel(output, context, W1, W2, Wout, bout):
    out, _ = run(dict(output=output, context=context, W1=W1, W2=W2,
                      Wout=Wout, bout=bout))
    return out
